# revision 19
# baseline (speedup 1.0000x reference)
"""ALIF spiking RNN forward on 8 TRN2 NeuronCores (Bass/Tile).

Strategy: data-parallel over batch (64 -> 8 per core). The scan over T=500
is strictly sequential; each step does the recurrent matmul
rec = W @ spikes_prev (1024x1024 weights, 8 batch rows per core) plus an
elementwise state update.

The matmul is weight-load bound on the PE (the full W streams through the
stationary path every step). fp32 matmul is ~8x slower than 16-bit on TRN2,
so weights are split W = Whi + Wlo in float16 (hi+lo carries ~22 mantissa
bits ~= fp32-exact; spikes are exactly representable) and accumulated in
fp32 PSUM - numerically fp32-faithful at 16-bit speed.

Layout: state tiles are [128 partitions = neuron%128, 64 cols = (neuron//128)*8
+ batch]. Spike tiles double as the matmul moving operand (cols J*8..J*8+8 are
exactly the J-th contraction tile) and as the DMA-out staging.

Per step the 128 matmuls run in 4 phases ((J in 0-3 / 4-7) x (g in 0-3 / 4-7))
accumulating into two PSUM banks, so each half of the spike-update DVE chain
overlaps the remaining matmul phases and the next step's matmuls start
immediately - step period ~= pure PE time.
"""
import numpy as np
import concourse.bass as bass
import concourse.bacc as bacc
import concourse.tile as tile
from concourse import mybir, bass_utils

F32 = mybir.dt.float32
F16 = mybir.dt.bfloat16  # spikes/weights dtype (bf16: no subnormal flush on Wlo)
ALU = mybir.AluOpType
ACTF = mybir.ActivationFunctionType

T, B, N = 500, 64, 1024
CORES = 8
BL = B // CORES          # 8 batch per core
G = N // 128             # 8 neuron tiles
COLS = G * BL            # 64 state columns
CH = 12                  # steps per loop body (even)
NITER = 41               # For_i iterations (41*12 = 492 steps)
REM = T - 2 - NITER * CH  # 6 remainder steps
XCOLS = T * COLS         # 32000

_CACHE = {}


def _build(dbg=False):
    key = ("nc", dbg)
    if key in _CACHE:
        return _CACHE[key]
    nc = bacc.Bacc("TRN2", target_bir_lowering=False, debug=False,
                   num_devices=CORES)
    x_d = nc.dram_tensor("x", [128, XCOLS], F32, kind="ExternalInput")
    whi_d = nc.dram_tensor("wt_hi", [N, N], F16, kind="ExternalInput")
    wlo_d = nc.dram_tensor("wt_lo", [N, N], F16, kind="ExternalInput")
    beta_d = nc.dram_tensor("beta_bc", [128, COLS], F32, kind="ExternalInput")
    p_d = nc.dram_tensor("p_bc", [128, COLS], F32, kind="ExternalInput")
    b_d = nc.dram_tensor("b_bc", [128, COLS], F32, kind="ExternalInput")
    out_d = nc.dram_tensor("out", [128, XCOLS], F16, kind="ExternalOutput")
    if dbg:
        nm_d = nc.dram_tensor("nm_out", [128, XCOLS], F32, kind="ExternalOutput")
        th_d = nc.dram_tensor("th_out", [128, XCOLS], F32, kind="ExternalOutput")

    with tile.TileContext(nc) as tc:
        import contextlib
        ctx = contextlib.ExitStack()
        with ctx:
            wpool = ctx.enter_context(tc.tile_pool(name="w", bufs=1))
            st = ctx.enter_context(tc.tile_pool(name="st", bufs=1))
            pspool = ctx.enter_context(tc.tile_pool(name="ps", bufs=1, space="PSUM"))

            # --- weights: [split][J] -> [128, 1024] fp16 (wt[j, i]) ---
            w_sb = [[wpool.tile([128, N], F16, tag=f"w{s}_{J}", name=f"w{s}_{J}")
                     for J in range(G)] for s in range(2)]
            for s, wd in enumerate((whi_d, wlo_d)):
                for J in range(G):
                    nc.sync.dma_start(w_sb[s][J][:], wd.ap()[J * 128:(J + 1) * 128, :])

            # --- parameter broadcast tiles ---
            beta_bc = st.tile([128, COLS], F32, tag="beta")
            p_bc = st.tile([128, COLS], F32, tag="p")
            b_bc = st.tile([128, COLS], F32, tag="b")
            nc.sync.dma_start(beta_bc[:], beta_d.ap())
            nc.sync.dma_start(p_bc[:], p_d.ap())
            nc.sync.dma_start(b_bc[:], b_d.ap())

            # --- state tiles (persistent, updated in place) ---
            mem = st.tile([128, COLS], F32, tag="mem")
            a_t = st.tile([128, COLS], F32, tag="a")
            rr = st.tile([128, COLS], F32, tag="rr")
            thr = st.tile([128, COLS], F32, tag="thr")
            notspk = st.tile([128, COLS], F32, tag="notspk")
            bm = st.tile([128, COLS], F32, tag="bm")
            ab = st.tile([128, COLS], F32, tag="ab")
            nm = st.tile([128, COLS], F32, tag="nm")
            nc.vector.memset(mem[:], 0.0)
            nc.vector.memset(a_t[:], 0.0)
            nc.vector.memset(rr[:], 2.0)
            nc.vector.memset(thr[:], 1.0)

            # --- staging ---
            xA = st.tile([128, 6 * COLS], F32, tag="xA")
            xB = st.tile([128, 6 * COLS], F32, tag="xB")
            oA = st.tile([128, 6 * COLS], F16, tag="oA")
            oB = st.tile([128, 6 * COLS], F16, tag="oB")
            x_pro = st.tile([128, 2 * COLS], F32, tag="xpro")
            o_pro = st.tile([128, 2 * COLS], F16, tag="opro")
            if dbg:
                nmA = st.tile([128, 6 * COLS], F32, tag="nmA")
                nmB = st.tile([128, 6 * COLS], F32, tag="nmB")
                thA = st.tile([128, 6 * COLS], F32, tag="thA")
                thB = st.tile([128, 6 * COLS], F32, tag="thB")
                nm_pro = st.tile([128, 2 * COLS], F32, tag="nmpro")
                th_pro = st.tile([128, 2 * COLS], F32, tag="thpro")
                dbg_map = {id(oA): (nmA, thA), id(oB): (nmB, thB),
                           id(o_pro): (nm_pro, th_pro)}

            # PSUM: two banks per parity (A = neuron groups 0-3, B = 4-7)
            psA = [pspool.tile([128, 32], F32, tag=f"psA{k}", name=f"psA{k}")
                   for k in range(2)]
            psB = [pspool.tile([128, 32], F32, tag=f"psB{k}", name=f"psB{k}")
                   for k in range(2)]

            pre = st.tile([128, COLS], F32, tag="pre")
            dmg = st.tile([128, COLS], F32, tag="dmg")
            mr = st.tile([128, COLS], F32, tag="mr")

            def estage(x_sl, first):
                """Early ops: depend only on previous step's state.
                pre = mem*beta + mask*x and d = thr - pre, so the critical
                chain after PSUM is just mask*rec -> cmp vs d."""
                if not first:
                    nc.vector.scalar_tensor_tensor(
                        rr[:], rr[:], 1.0, notspk[:], ALU.add, ALU.mult)
                nc.vector.tensor_tensor(bm[:], mem[:], beta_bc[:], ALU.mult)
                nc.vector.tensor_tensor(ab[:], a_t[:], b_bc[:], ALU.mult)
                nc.scalar.activation(thr[:], ab[:], ACTF.Copy, bias=1.0, scale=1.0)
                nc.vector.scalar_tensor_tensor(
                    pre[:], rr[:], 2.0, x_sl, ALU.is_ge, ALU.mult)
                nc.vector.tensor_tensor(pre[:], bm[:], pre[:], ALU.add)
                nc.vector.tensor_tensor(dmg[:], thr[:], pre[:], ALU.subtract)

            def cstage(h, ps, spk_out, after=None):
                """Critical half-chain: mask*rec -> spike (2 ops).
                `after`: scheduling-order-only dep so half B's chain doesn't
                interleave ahead of half A's on the DVE queue."""
                from concourse.tile_rust import add_dep_helper
                c = slice(32 * h, 32 * (h + 1))
                i1 = nc.vector.scalar_tensor_tensor(
                    mr[:, c], rr[:, c], 2.0, ps[:], ALU.is_ge, ALU.mult)
                if after is not None:
                    add_dep_helper(i1.ins, after.ins, sync=False,
                                   reason="keep half-B chain after half-A")
                last = nc.vector.tensor_tensor(spk_out, mr[:, c], dmg[:, c], ALU.is_gt)
                return last

            def cstage_pro(h, spk_out):
                """Prologue variant: no recurrent input (mask*rec = 0)."""
                c = slice(32 * h, 32 * (h + 1))
                nc.vector.memset(mr[:, c], 0.0)
                nc.vector.tensor_tensor(spk_out, mr[:, c], dmg[:, c], ALU.is_gt)

            def pstage(spk16):
                """Post ops: state update from the new spikes."""
                nc.scalar.activation(notspk[:], spk16, ACTF.Copy, bias=1.0, scale=-1.0)
                nc.vector.tensor_tensor(nm[:], pre[:], mr[:], ALU.add)
                nc.vector.tensor_tensor(mem[:], nm[:], notspk[:], ALU.mult)
                nc.vector.tensor_tensor(ab[:], a_t[:], p_bc[:], ALU.mult)
                nc.vector.tensor_tensor(a_t[:], ab[:], spk16, ALU.add)

            def matmuls(par, rhs_tile, rhs_col):
                """128 matmuls of one step in 4 phases; rhs = previous spikes."""
                pA, pB = psA[par], psB[par]
                for phase in range(4):
                    Js = range(0, 4) if phase < 2 else range(4, 8)
                    gs = range(0, 4) if phase % 2 == 0 else range(4, 8)
                    ps = pA if phase % 2 == 0 else pB
                    for J in Js:
                        rhs = rhs_tile[:, rhs_col + J * BL: rhs_col + (J + 1) * BL]
                        for g in gs:
                            for s in range(2):
                                # start=True clears has_written for the WHOLE
                                # bank -> exactly one start/stop per bank epoch;
                                # later first-writes to a region overwrite+set.
                                nc.tensor.matmul(
                                    ps[:, (g % 4) * BL:(g % 4 + 1) * BL],
                                    w_sb[s][J][:, g * 128:(g + 1) * 128],
                                    rhs,
                                    start=(phase < 2 and J == 0
                                           and g == gs[0] and s == 0),
                                    stop=(phase >= 2 and J == 7
                                          and g == gs[-1] and s == 1),
                                    skip_group_check=True,
                                )

            def step(par, x_tile, sl, o_tile, prev_tile, prev_s, first=False):
                """One timestep. par: psum parity; sl: local slot in x/o tile;
                prev_tile/prev_s: where the previous step's spikes live."""
                xs = x_tile[:, sl * COLS:(sl + 1) * COLS]
                estage(xs, first)
                if prev_tile is None:
                    for h in range(2):
                        spk_out = o_tile[:, sl * COLS + 32 * h: sl * COLS + 32 * (h + 1)]
                        cstage_pro(h, spk_out)
                else:
                    matmuls(par, prev_tile, prev_s * COLS)
                    last = None
                    for h in range(2):
                        spk_out = o_tile[:, sl * COLS + 32 * h: sl * COLS + 32 * (h + 1)]
                        last = cstage(h, (psA if h == 0 else psB)[par], spk_out,
                                      after=last)
                if dbg:
                    nmt, tht = dbg_map[id(o_tile)]
                    nc.vector.tensor_copy(nmt[:, sl * COLS:(sl + 1) * COLS], nm[:])
                    nc.vector.tensor_copy(tht[:, sl * COLS:(sl + 1) * COLS], thr[:])
                pstage(o_tile[:, sl * COLS:(sl + 1) * COLS])

            # ---------- prologue: t = 0, 1 (no recurrent input) ----------
            nc.sync.dma_start(x_pro[:], x_d.ap()[:, 0:2 * COLS])
            nc.sync.dma_start(xA[:], x_d.ap()[:, 2 * COLS:8 * COLS])
            step(0, x_pro, 0, o_pro, None, 0, first=True)
            step(1, x_pro, 1, o_pro, None, 0)
            # t=1 spikes also feed the first main step's matmuls
            nc.vector.tensor_copy(oB[:, 5 * COLS:6 * COLS],
                                  o_pro[:, 1 * COLS:2 * COLS])
            nc.sync.dma_start(out_d.ap()[:, XCOLS - 2 * COLS:XCOLS], o_pro[:])
            if dbg:
                nc.sync.dma_start(nm_d.ap()[:, XCOLS - 2 * COLS:XCOLS], nm_pro[:])
                nc.sync.dma_start(th_d.ap()[:, XCOLS - 2 * COLS:XCOLS], th_pro[:])

            # ---------- main loop: t = 2 .. 493 ----------
            with tc.For_i(0, NITER * CH * COLS, CH * COLS,
                          hint_engines=(mybir.EngineType.PE,)) as off:
                # prefetch x for steps 6-11 of this iteration
                nc.sync.dma_start(xB[:], x_d.ap()[:, bass.ds(off + 8 * COLS, 6 * COLS)])
                step(0, xA, 0, oA, oB, 5)
                for s in range(1, 6):
                    step(s % 2, xA, s, oA, oA, s - 1)
                nc.sync.dma_start(out_d.ap()[:, bass.ds(off, 6 * COLS)], oA[:])
                if dbg:
                    nc.sync.dma_start(nm_d.ap()[:, bass.ds(off, 6 * COLS)], nmA[:])
                    nc.sync.dma_start(th_d.ap()[:, bass.ds(off, 6 * COLS)], thA[:])
                # prefetch x for steps 0-5 of the NEXT iteration
                nc.sync.dma_start(xA[:], x_d.ap()[:, bass.ds(off + 14 * COLS, 6 * COLS)])
                step(0, xB, 0, oB, oA, 5)
                for s in range(7, 12):
                    step(s % 2, xB, s - 6, oB, oB, s - 7)
                nc.sync.dma_start(out_d.ap()[:, bass.ds(off + 6 * COLS, 6 * COLS)], oB[:])
                if dbg:
                    nc.sync.dma_start(nm_d.ap()[:, bass.ds(off + 6 * COLS, 6 * COLS)], nmB[:])
                    nc.sync.dma_start(th_d.ap()[:, bass.ds(off + 6 * COLS, 6 * COLS)], thB[:])

            # ---------- remainder: t = 494 .. 499 (x already in xA) ----------
            step(0, xA, 0, oA, oB, 5)
            for s in range(1, REM):
                step(s % 2, xA, s, oA, oA, s - 1)
            main_cols = NITER * CH * COLS
            nc.sync.dma_start(out_d.ap()[:, main_cols:main_cols + REM * COLS],
                              oA[:, 0:REM * COLS])
            if dbg:
                nc.sync.dma_start(nm_d.ap()[:, main_cols:main_cols + REM * COLS],
                                  nmA[:, 0:REM * COLS])
                nc.sync.dma_start(th_d.ap()[:, main_cols:main_cols + REM * COLS],
                                  thA[:, 0:REM * COLS])

    nc.compile()
    _CACHE[key] = nc
    return nc


def _prep_inputs(x, rec_weight, beta_param, p_param, b_param):
    x = np.asarray(x, dtype=np.float32)
    W = np.asarray(rec_weight, dtype=np.float32)
    beta = np.clip(np.asarray(beta_param, dtype=np.float32), 0.001, 0.999)
    p = np.clip(np.abs(np.asarray(p_param, dtype=np.float32)), 0.0, 0.999)
    b = np.clip(np.abs(np.asarray(b_param, dtype=np.float32)), 0.001, 1.0)

    import ml_dtypes
    Whi = W.astype(ml_dtypes.bfloat16)
    Wlo = (W - Whi.astype(np.float32)).astype(ml_dtypes.bfloat16)
    wt_hi = np.ascontiguousarray(Whi.T)   # [j, i]
    wt_lo = np.ascontiguousarray(Wlo.T)

    def bc(v):  # [N] -> [128, COLS] with col = g*BL + b
        m = v.reshape(G, 128).T            # [128, G]
        return np.ascontiguousarray(np.repeat(m, BL, axis=1))

    in_maps = []
    for c in range(CORES):
        xc = x[:, c * BL:(c + 1) * BL, :]                 # [T, BL, N]
        xc = xc.reshape(T, BL, G, 128).transpose(3, 0, 2, 1)  # [128, T, G, BL]
        xc = np.ascontiguousarray(xc.reshape(128, XCOLS))
        in_maps.append({
            "x": xc,
            "wt_hi": wt_hi,
            "wt_lo": wt_lo,
            "beta_bc": bc(beta),
            "p_bc": bc(p),
            "b_bc": bc(b),
        })
    return in_maps


def _unshard(results):
    # device col layout: [0 : 492*COLS) -> t=2..493, then t=494..499, then t=0,1
    order = np.empty(T, dtype=np.int64)
    order[2:494] = np.arange(0, 492)
    order[494:500] = np.arange(492, 498)
    order[0:2] = np.arange(498, 500)
    out = np.empty((T, B, N), dtype=np.float32)
    for c in range(CORES):
        oc = results[c]["out"].astype(np.float32)      # [128, XCOLS]
        oc = oc.reshape(128, T, G, BL)                 # [p, tslot, g, b]
        oc = oc[:, order, :, :]                        # -> t order
        # out[t, b, g*128+p] = oc[p, t, g, b]
        out[:, c * BL:(c + 1) * BL, :] = oc.transpose(1, 3, 2, 0).reshape(T, BL, N)
    return out


def run(inputs, trace=False, dbg=False):
    nc = _build(dbg)
    in_maps = _prep_inputs(**inputs)
    res = bass_utils.run_bass_kernel_spmd(
        nc, in_maps, core_ids=list(range(CORES)), trace=trace)
    return _unshard(res.results), res


def kernel(**inputs):
    out, _ = run(inputs)
    return out


# revision 20
# speedup vs baseline: 1.1963x; 1.1963x over previous
"""ALIF spiking RNN forward on 8 TRN2 NeuronCores (Bass/Tile).

Strategy: data-parallel over batch (64 -> 8 per core). The scan over T=500
is strictly sequential; each step does the recurrent matmul
rec = W @ spikes_prev (1024x1024 weights, 8 batch rows per core) plus an
elementwise state update.

The matmul is weight-load bound on the PE (the full W streams through the
stationary path every step). fp32 matmul is ~8x slower than 16-bit on TRN2,
so weights are split W = Whi + Wlo in float16 (hi+lo carries ~22 mantissa
bits ~= fp32-exact; spikes are exactly representable) and accumulated in
fp32 PSUM - numerically fp32-faithful at 16-bit speed.

Layout: state tiles are [128 partitions = neuron%128, 64 cols = (neuron//128)*8
+ batch]. Spike tiles double as the matmul moving operand (cols J*8..J*8+8 are
exactly the J-th contraction tile) and as the DMA-out staging.

Per step the 128 matmuls run in 4 phases ((J in 0-3 / 4-7) x (g in 0-3 / 4-7))
accumulating into two PSUM banks, so each half of the spike-update DVE chain
overlaps the remaining matmul phases and the next step's matmuls start
immediately - step period ~= pure PE time.
"""
import numpy as np
import concourse.bass as bass
import concourse.bacc as bacc
import concourse.tile as tile
from concourse import mybir, bass_utils

F32 = mybir.dt.float32
F16 = mybir.dt.bfloat16  # spikes/weights dtype (bf16: no subnormal flush on Wlo)
ALU = mybir.AluOpType
ACTF = mybir.ActivationFunctionType

T, B, N = 500, 64, 1024
CORES = 8
BL = B // CORES          # 8 batch per core
G = N // 128             # 8 neuron tiles
COLS = G * BL            # 64 state columns
CH = 12                  # steps per loop body (even)
NITER = 41               # For_i iterations (41*12 = 492 steps)
REM = T - 2 - NITER * CH  # 6 remainder steps
XCOLS = T * COLS         # 32000

_CACHE = {}


def _build(dbg=False):
    key = ("nc", dbg)
    if key in _CACHE:
        return _CACHE[key]
    nc = bacc.Bacc("TRN2", target_bir_lowering=False, debug=False,
                   num_devices=CORES)
    x_d = nc.dram_tensor("x", [128, XCOLS], F32, kind="ExternalInput")
    whi_d = nc.dram_tensor("wt_hi", [N, N], F16, kind="ExternalInput")
    wlo_d = nc.dram_tensor("wt_lo", [N, N], F16, kind="ExternalInput")
    beta_d = nc.dram_tensor("beta_bc", [128, COLS], F32, kind="ExternalInput")
    p_d = nc.dram_tensor("p_bc", [128, COLS], F32, kind="ExternalInput")
    b_d = nc.dram_tensor("b_bc", [128, COLS], F32, kind="ExternalInput")
    out_d = nc.dram_tensor("out", [128, XCOLS], F16, kind="ExternalOutput")
    if dbg:
        nm_d = nc.dram_tensor("nm_out", [128, XCOLS], F32, kind="ExternalOutput")
        th_d = nc.dram_tensor("th_out", [128, XCOLS], F32, kind="ExternalOutput")

    with tile.TileContext(nc) as tc:
        import contextlib
        ctx = contextlib.ExitStack()
        with ctx:
            wpool = ctx.enter_context(tc.tile_pool(name="w", bufs=1))
            st = ctx.enter_context(tc.tile_pool(name="st", bufs=1))
            pspool = ctx.enter_context(tc.tile_pool(name="ps", bufs=1, space="PSUM"))

            # --- weights: [split][J] -> [128, 1024] fp16 (wt[j, i]) ---
            w_sb = [[wpool.tile([128, N], F16, tag=f"w{s}_{J}", name=f"w{s}_{J}")
                     for J in range(G)] for s in range(2)]
            for s, wd in enumerate((whi_d, wlo_d)):
                for J in range(G):
                    nc.sync.dma_start(w_sb[s][J][:], wd.ap()[J * 128:(J + 1) * 128, :])

            # --- parameter broadcast tiles ---
            beta_bc = st.tile([128, COLS], F32, tag="beta")
            p_bc = st.tile([128, COLS], F32, tag="p")
            b_bc = st.tile([128, COLS], F32, tag="b")
            nc.sync.dma_start(beta_bc[:], beta_d.ap())
            nc.sync.dma_start(p_bc[:], p_d.ap())
            nc.sync.dma_start(b_bc[:], b_d.ap())

            # --- state tiles (persistent, updated in place) ---
            mem = st.tile([128, COLS], F32, tag="mem")
            a_t = st.tile([128, COLS], F32, tag="a")
            rr = st.tile([128, COLS], F32, tag="rr")
            thr = st.tile([128, COLS], F32, tag="thr")
            notspk = st.tile([128, COLS], F32, tag="notspk")
            bm = st.tile([128, COLS], F32, tag="bm")
            ab = st.tile([128, COLS], F32, tag="ab")
            nm = st.tile([128, COLS], F32, tag="nm")
            nc.vector.memset(mem[:], 0.0)
            nc.vector.memset(a_t[:], 0.0)
            nc.vector.memset(rr[:], 2.0)
            nc.vector.memset(thr[:], 1.0)

            # --- staging ---
            xA = st.tile([128, 6 * COLS], F32, tag="xA")
            xB = st.tile([128, 6 * COLS], F32, tag="xB")
            oA = st.tile([128, 6 * COLS], F16, tag="oA")
            oB = st.tile([128, 6 * COLS], F16, tag="oB")
            x_pro = st.tile([128, 2 * COLS], F32, tag="xpro")
            o_pro = st.tile([128, 2 * COLS], F16, tag="opro")
            if dbg:
                nmA = st.tile([128, 6 * COLS], F32, tag="nmA")
                nmB = st.tile([128, 6 * COLS], F32, tag="nmB")
                thA = st.tile([128, 6 * COLS], F32, tag="thA")
                thB = st.tile([128, 6 * COLS], F32, tag="thB")
                nm_pro = st.tile([128, 2 * COLS], F32, tag="nmpro")
                th_pro = st.tile([128, 2 * COLS], F32, tag="thpro")
                dbg_map = {id(oA): (nmA, thA), id(oB): (nmB, thB),
                           id(o_pro): (nm_pro, th_pro)}

            # PSUM: two banks per parity (A = neuron groups 0-3, B = 4-7)
            psA = [pspool.tile([128, 32], F32, tag=f"psA{k}", name=f"psA{k}")
                   for k in range(2)]
            psB = [pspool.tile([128, 32], F32, tag=f"psB{k}", name=f"psB{k}")
                   for k in range(2)]

            pre = st.tile([128, COLS], F32, tag="pre")
            dmg = st.tile([128, COLS], F32, tag="dmg")
            mr = st.tile([128, COLS], F32, tag="mr")

            def estage(x_sl, first):
                """Early ops: depend only on previous step's state.
                pre = mem*beta + mask*x and d = thr - pre, so the critical
                chain after PSUM is just mask*rec -> cmp vs d."""
                if not first:
                    nc.vector.scalar_tensor_tensor(
                        rr[:], rr[:], 1.0, notspk[:], ALU.add, ALU.mult)
                nc.vector.tensor_tensor(bm[:], mem[:], beta_bc[:], ALU.mult)
                nc.vector.tensor_tensor(ab[:], a_t[:], b_bc[:], ALU.mult)
                nc.vector.scalar_tensor_tensor(
                    pre[:], rr[:], 2.0, x_sl, ALU.is_ge, ALU.mult)
                nc.vector.tensor_tensor(pre[:], bm[:], pre[:], ALU.add)
                # d = (b*a + 1) - pre  == thr - pre
                nc.vector.scalar_tensor_tensor(
                    dmg[:], ab[:], 1.0, pre[:], ALU.add, ALU.subtract)

            def cstage(h, ps, spk_out, after=None):
                """Critical half-chain: mask*rec -> spike (2 ops).
                `after`: scheduling-order-only dep so half B's chain doesn't
                interleave ahead of half A's on the DVE queue."""
                from concourse.tile_rust import add_dep_helper
                c = slice(32 * h, 32 * (h + 1))
                i1 = nc.vector.scalar_tensor_tensor(
                    mr[:, c], rr[:, c], 2.0, ps[:], ALU.is_ge, ALU.mult)
                if after is not None:
                    add_dep_helper(i1.ins, after.ins, sync=False,
                                   reason="keep half-B chain after half-A")
                last = nc.vector.tensor_tensor(spk_out, mr[:, c], dmg[:, c], ALU.is_gt)
                return last

            def cstage_pro(h, spk_out):
                """Prologue variant: no recurrent input (mask*rec = 0)."""
                c = slice(32 * h, 32 * (h + 1))
                nc.vector.memset(mr[:, c], 0.0)
                nc.vector.tensor_tensor(spk_out, mr[:, c], dmg[:, c], ALU.is_gt)

            def pstage(spk16):
                """Post ops: state update from the new spikes."""
                nc.scalar.activation(notspk[:], spk16, ACTF.Copy, bias=1.0, scale=-1.0)
                nc.vector.tensor_tensor(nm[:], pre[:], mr[:], ALU.add)
                nc.vector.tensor_tensor(mem[:], nm[:], notspk[:], ALU.mult)
                nc.vector.tensor_tensor(ab[:], a_t[:], p_bc[:], ALU.mult)
                nc.vector.tensor_tensor(a_t[:], ab[:], spk16, ALU.add)

            def matmuls(par, rhs_tile, rhs_col):
                """128 matmuls of one step in 4 phases; rhs = previous spikes."""
                pA, pB = psA[par], psB[par]
                for phase in range(4):
                    Js = range(0, 4) if phase < 2 else range(4, 8)
                    gs = range(0, 4) if phase % 2 == 0 else range(4, 8)
                    ps = pA if phase % 2 == 0 else pB
                    for J in Js:
                        rhs = rhs_tile[:, rhs_col + J * BL: rhs_col + (J + 1) * BL]
                        for g in gs:
                            for s in range(2):
                                # start=True clears has_written for the WHOLE
                                # bank -> exactly one start/stop per bank epoch;
                                # later first-writes to a region overwrite+set.
                                nc.tensor.matmul(
                                    ps[:, (g % 4) * BL:(g % 4 + 1) * BL],
                                    w_sb[s][J][:, g * 128:(g + 1) * 128],
                                    rhs,
                                    start=(phase < 2 and J == 0
                                           and g == gs[0] and s == 0),
                                    stop=(phase >= 2 and J == 7
                                          and g == gs[-1] and s == 1),
                                    skip_group_check=True,
                                )

            def step(par, x_tile, sl, o_tile, prev_tile, prev_s, first=False):
                """One timestep. par: psum parity; sl: local slot in x/o tile;
                prev_tile/prev_s: where the previous step's spikes live."""
                xs = x_tile[:, sl * COLS:(sl + 1) * COLS]
                estage(xs, first)
                if prev_tile is None:
                    for h in range(2):
                        spk_out = o_tile[:, sl * COLS + 32 * h: sl * COLS + 32 * (h + 1)]
                        cstage_pro(h, spk_out)
                else:
                    matmuls(par, prev_tile, prev_s * COLS)
                    last = None
                    for h in range(2):
                        spk_out = o_tile[:, sl * COLS + 32 * h: sl * COLS + 32 * (h + 1)]
                        last = cstage(h, (psA if h == 0 else psB)[par], spk_out,
                                      after=last)
                if dbg:
                    nmt, tht = dbg_map[id(o_tile)]
                    nc.vector.tensor_copy(nmt[:, sl * COLS:(sl + 1) * COLS], nm[:])
                    nc.vector.tensor_copy(tht[:, sl * COLS:(sl + 1) * COLS], thr[:])
                pstage(o_tile[:, sl * COLS:(sl + 1) * COLS])

            # ---------- prologue: t = 0, 1 (no recurrent input) ----------
            nc.sync.dma_start(x_pro[:], x_d.ap()[:, 0:2 * COLS])
            nc.sync.dma_start(xA[:], x_d.ap()[:, 2 * COLS:8 * COLS])
            step(0, x_pro, 0, o_pro, None, 0, first=True)
            step(1, x_pro, 1, o_pro, None, 0)
            # t=1 spikes also feed the first main step's matmuls
            nc.vector.tensor_copy(oB[:, 5 * COLS:6 * COLS],
                                  o_pro[:, 1 * COLS:2 * COLS])
            nc.sync.dma_start(out_d.ap()[:, XCOLS - 2 * COLS:XCOLS], o_pro[:])
            if dbg:
                nc.sync.dma_start(nm_d.ap()[:, XCOLS - 2 * COLS:XCOLS], nm_pro[:])
                nc.sync.dma_start(th_d.ap()[:, XCOLS - 2 * COLS:XCOLS], th_pro[:])

            # ---------- main loop: t = 2 .. 493 ----------
            with tc.For_i(0, NITER * CH * COLS, CH * COLS,
                          hint_engines=(mybir.EngineType.PE,)) as off:
                # prefetch x for steps 6-11 of this iteration
                nc.sync.dma_start(xB[:], x_d.ap()[:, bass.ds(off + 8 * COLS, 6 * COLS)])
                step(0, xA, 0, oA, oB, 5)
                for s in range(1, 6):
                    step(s % 2, xA, s, oA, oA, s - 1)
                nc.sync.dma_start(out_d.ap()[:, bass.ds(off, 6 * COLS)], oA[:])
                if dbg:
                    nc.sync.dma_start(nm_d.ap()[:, bass.ds(off, 6 * COLS)], nmA[:])
                    nc.sync.dma_start(th_d.ap()[:, bass.ds(off, 6 * COLS)], thA[:])
                # prefetch x for steps 0-5 of the NEXT iteration
                nc.sync.dma_start(xA[:], x_d.ap()[:, bass.ds(off + 14 * COLS, 6 * COLS)])
                step(0, xB, 0, oB, oA, 5)
                for s in range(7, 12):
                    step(s % 2, xB, s - 6, oB, oB, s - 7)
                nc.sync.dma_start(out_d.ap()[:, bass.ds(off + 6 * COLS, 6 * COLS)], oB[:])
                if dbg:
                    nc.sync.dma_start(nm_d.ap()[:, bass.ds(off + 6 * COLS, 6 * COLS)], nmB[:])
                    nc.sync.dma_start(th_d.ap()[:, bass.ds(off + 6 * COLS, 6 * COLS)], thB[:])

            # ---------- remainder: t = 494 .. 499 (x already in xA) ----------
            step(0, xA, 0, oA, oB, 5)
            for s in range(1, REM):
                step(s % 2, xA, s, oA, oA, s - 1)
            main_cols = NITER * CH * COLS
            nc.sync.dma_start(out_d.ap()[:, main_cols:main_cols + REM * COLS],
                              oA[:, 0:REM * COLS])
            if dbg:
                nc.sync.dma_start(nm_d.ap()[:, main_cols:main_cols + REM * COLS],
                                  nmA[:, 0:REM * COLS])
                nc.sync.dma_start(th_d.ap()[:, main_cols:main_cols + REM * COLS],
                                  thA[:, 0:REM * COLS])

    nc.compile()
    _CACHE[key] = nc
    return nc


def _prep_inputs(x, rec_weight, beta_param, p_param, b_param):
    x = np.asarray(x, dtype=np.float32)
    W = np.asarray(rec_weight, dtype=np.float32)
    beta = np.clip(np.asarray(beta_param, dtype=np.float32), 0.001, 0.999)
    p = np.clip(np.abs(np.asarray(p_param, dtype=np.float32)), 0.0, 0.999)
    b = np.clip(np.abs(np.asarray(b_param, dtype=np.float32)), 0.001, 1.0)

    import ml_dtypes
    Whi = W.astype(ml_dtypes.bfloat16)
    Wlo = (W - Whi.astype(np.float32)).astype(ml_dtypes.bfloat16)
    wt_hi = np.ascontiguousarray(Whi.T)   # [j, i]
    wt_lo = np.ascontiguousarray(Wlo.T)

    def bc(v):  # [N] -> [128, COLS] with col = g*BL + b
        m = v.reshape(G, 128).T            # [128, G]
        return np.ascontiguousarray(np.repeat(m, BL, axis=1))

    in_maps = []
    for c in range(CORES):
        xc = x[:, c * BL:(c + 1) * BL, :]                 # [T, BL, N]
        xc = xc.reshape(T, BL, G, 128).transpose(3, 0, 2, 1)  # [128, T, G, BL]
        xc = np.ascontiguousarray(xc.reshape(128, XCOLS))
        in_maps.append({
            "x": xc,
            "wt_hi": wt_hi,
            "wt_lo": wt_lo,
            "beta_bc": bc(beta),
            "p_bc": bc(p),
            "b_bc": bc(b),
        })
    return in_maps


def _unshard(results):
    # device col layout: [0 : 492*COLS) -> t=2..493, then t=494..499, then t=0,1
    order = np.empty(T, dtype=np.int64)
    order[2:494] = np.arange(0, 492)
    order[494:500] = np.arange(492, 498)
    order[0:2] = np.arange(498, 500)
    out = np.empty((T, B, N), dtype=np.float32)
    for c in range(CORES):
        oc = results[c]["out"].astype(np.float32)      # [128, XCOLS]
        oc = oc.reshape(128, T, G, BL)                 # [p, tslot, g, b]
        oc = oc[:, order, :, :]                        # -> t order
        # out[t, b, g*128+p] = oc[p, t, g, b]
        out[:, c * BL:(c + 1) * BL, :] = oc.transpose(1, 3, 2, 0).reshape(T, BL, N)
    return out


def run(inputs, trace=False, dbg=False):
    nc = _build(dbg)
    in_maps = _prep_inputs(**inputs)
    res = bass_utils.run_bass_kernel_spmd(
        nc, in_maps, core_ids=list(range(CORES)), trace=trace)
    return _unshard(res.results), res


def kernel(**inputs):
    out, _ = run(inputs)
    return out
